# revision 18
# baseline (speedup 1.0000x reference)
"""Distributed causal multi-head attention for Trainium2 (8 NeuronCores).

Reference computes, for x [2, 2048, 1024]:
    qkv = x @ W_qkv + b_qkv ; split into q,k,v heads (16 heads, d_k=64)
    causal softmax attention per head
    out = ctx @ W_o + b_o

Sharding (data + head parallel): core c handles batch b=c//4 and heads
H = [4g..4g+3] with g=c%4.  Each core:
  - computes q^T,k^T ([dk, T] layout, head-pairs packed into 128 partitions)
    and v ([T, dk] natural layout, augmented with a ones column so the
    attention-weights matmul also produces softmax denominators),
  - computes its heads' causal T x T attention, normalizes ctx^T into SBUF
    ([128, T] bf16 per head pair),
  - computes a full-width partial output projection from its own 4 heads
    (out_partial[T, 1024] = ctx_local @ W_o[local rows, :]) in bf16,
  - ReduceScatters the partials within its 4-core batch group per 512-row
    q-chunk; each core ends with 4 x [128, 1024] summed rows, adds bias and
    stores [512, 1024] f32.
Host-side: shard prep (pack/transpose/bf16-cast) and reassembly of the
row-slices.  All FLOPs (matmuls, softmax, reductions) on device.
"""

import numpy as np
import ml_dtypes

import concourse.bass as bass
import concourse.mybir as mybir
import concourse.tile as tile
from concourse import bacc
from concourse import bass_utils

BF16 = mybir.dt.bfloat16
F32 = mybir.dt.float32
AF = mybir.ActivationFunctionType

T = 2048
D = 1024
NH = 16
HPC = 4   # heads per core
DK = 64
NCORES = 8
TQ = 512  # q-chunk (free dim of logits^T tiles)
NQC = T // TQ   # 4
NDT = D // 128  # 8 d-tiles
NTT = T // 128  # 16 t-tiles
VW = DK + 1     # 65: v columns per head incl. ones column
WV = HPC * VW   # 260
SCALE = 1.0 / 8.0  # 1/sqrt(DK)

TRACE = False
DEBUG = False  # adds intermediate-dump outputs (debug.py only)
LAST_RESULT = {}

_cache = {}


def _build():
    nc = bacc.Bacc("TRN2", target_bir_lowering=False, debug=False,
                   num_devices=NCORES)

    # host-packed inputs ([128, N] layouts; see _shard_inputs)
    xt = nc.declare_dram_parameter("xt", [128, 4 * NQC * TQ * 2], BF16, False)
    wq = nc.declare_dram_parameter("wq", [128, NDT * 256], BF16, False)
    wk = nc.declare_dram_parameter("wk", [128, NDT * 256], BF16, False)
    wv = nc.declare_dram_parameter("wv", [128, NDT * WV], BF16, False)
    wo = nc.declare_dram_parameter("wo", [128, 2 * D], BF16, False)
    bq = nc.declare_dram_parameter("bq", [128, 2], F32, False)
    bk = nc.declare_dram_parameter("bk", [128, 2], F32, False)
    bv = nc.declare_dram_parameter("bv", [128, WV], F32, False)
    bo = nc.declare_dram_parameter("bo", [128, D], F32, False)
    masks = nc.declare_dram_parameter("masks", [128, 4 * TQ], BF16, False)
    out = nc.declare_dram_parameter("out", [NQC * 128, D], F32, True)

    # proj partials + ReduceScatter buffers
    partial = nc.dram_tensor("partial", [T, D], BF16)
    rs_out = [nc.dram_tensor(f"rs_out{q}", [128, D], BF16)
              for q in range(NQC)]

    dbg = {}
    if DEBUG:
        for nm, shp in [("dbg_qT", [128, 2 * T]), ("dbg_kT", [128, 2 * T]),
                        ("dbg_v", [128, NTT * WV]),
                        ("dbg_ctxn0", [128, T]), ("dbg_ctxn1", [128, T]),
                        ("dbg_partial", [T, D]),
                        ("dbg_rs", [NQC * 128, D])]:
            dbg[nm] = nc.declare_dram_parameter(nm, shp, BF16, True)

    with tile.TileContext(nc) as tc, tc.tile_pool(name="pers", bufs=1) as pers:
        # ---------------- persistent SBUF ----------------
        xt_sb = pers.tile([128, NDT * T], BF16, tag="xt_sb", name="xt_sb")
        wq_sb = pers.tile([128, NDT * 256], BF16, tag="wq_sb", name="wq_sb")
        wk_sb = pers.tile([128, NDT * 256], BF16, tag="wk_sb", name="wk_sb")
        wv_sb = pers.tile([128, NDT * WV], BF16, tag="wv_sb", name="wv_sb")
        wo_sb = pers.tile([128, 2 * D], BF16, tag="wo_sb", name="wo_sb")
        bq_sb = pers.tile([128, 2], F32, tag="bq_sb", name="bq_sb")
        bk_sb = pers.tile([128, 2], F32, tag="bk_sb", name="bk_sb")
        bv_sb = pers.tile([128, WV], F32, tag="bv_sb", name="bv_sb")
        bo_sb = pers.tile([128, D], F32, tag="bo_sb", name="bo_sb")
        mask_sb = pers.tile([128, 4 * TQ], BF16, tag="mask_sb", name="mask_sb")
        qT_sb = pers.tile([128, 2 * T], BF16, tag="qT_sb", name="qT_sb")
        kT_sb = pers.tile([128, 2 * T], BF16, tag="kT_sb", name="kT_sb")
        v_sb = pers.tile([128, NTT * WV], BF16, tag="v_sb", name="v_sb")
        # normalized ctx^T per head pair: rows 0-63 head 2p, 64-127 head 2p+1
        ctxn = [pers.tile([128, T], BF16, tag=f"ctxn{p}", name=f"ctxn{p}")
                for p in range(2)]
        ones1 = pers.tile([1, DK], F32, tag="ones1", name="ones1")
        nc.gpsimd.memset(ones1[:], 1.0)

        # ---------------- input DMAs (sync + scalar queues) -----------
        # xt layout: [128, qc*4096 + d*512 + c]; issue per (qc, d-pair)
        # so 16 parallel DMA engines carry ~256KB each.  Both HWDGE queues
        # (sync, scalar) issue in earliest-needed order: qc0's QK deps
        # first, then v deps, then later chunks.
        def xt_piece(qc, dp):
            c0 = qc * (NDT * TQ) + dp * 1024
            return (xt_sb[:, c0:c0 + 1024], xt[:, c0:c0 + 1024])

        sync_q, scal_q = [], []
        sync_q.append((bq_sb[:], bq[:]))
        scal_q.append((bk_sb[:], bk[:]))
        # qc0 critical path: split into 128KB pieces across both queues
        for i in range(4):
            c0 = i * 512
            (sync_q if i % 2 == 0 else scal_q).append(
                (wq_sb[:, c0:c0 + 512], wq[:, c0:c0 + 512]))
            (scal_q if i % 2 == 0 else sync_q).append(
                (wk_sb[:, c0:c0 + 512], wk[:, c0:c0 + 512]))
        for dp in range(4):
            c0 = dp * 1024
            (sync_q if dp % 2 == 0 else scal_q).append(
                (xt_sb[:, c0:c0 + 512], xt[:, c0:c0 + 512]))
            (scal_q if dp % 2 == 0 else sync_q).append(
                (xt_sb[:, c0 + 512:c0 + 1024], xt[:, c0 + 512:c0 + 1024]))
        for i in range(2):
            sync_q.append((wv_sb[:, i * 1040:(i + 1) * 1040],
                           wv[:, i * 1040:(i + 1) * 1040]))
            scal_q.append((mask_sb[:, i * 1024:(i + 1) * 1024],
                           masks[:, i * 1024:(i + 1) * 1024]))
        sync_q.append((bv_sb[:], bv[:]))
        for qc in range(1, NQC):
            sync_q += [xt_piece(qc, 0), xt_piece(qc, 2)]
            scal_q += [xt_piece(qc, 1), xt_piece(qc, 3)]
        for i in range(2):
            c0 = i * 1024
            scal_q.append((wo_sb[:, c0:c0 + 1024], wo[:, c0:c0 + 1024]))
            sync_q.append((bo_sb[:, i * 512:(i + 1) * 512],
                           bo[:, i * 512:(i + 1) * 512]))
        for dst, src in sync_q:
            nc.sync.dma_start(dst, src)
        for dst, src in scal_q:
            nc.scalar.dma_start(dst, src)

        def xt_ap(d, tlo, thi):
            # xt_sb columns for d-tile d, t-range [tlo, thi)
            qc = tlo // TQ
            off = qc * (NDT * TQ) + d * TQ + (tlo - qc * TQ)
            return xt_sb[:, off:off + (thi - tlo)]

        with (
            tc.tile_pool(name="pp", space="PSUM", bufs=2) as pp,
            tc.tile_pool(name="sp", space="SBUF", bufs=2) as sp,
        ):
            # ---------------- filler emitters (QKV proj groups) -----------
            # each emits one PSUM-tile group; pumped between attention
            # groups to fill PE gaps during exp waits.  PSUM budget
            # (8 banks): lgX 2 + lgY 2 + ctxX 1 + ctxY 1 + fil 2.
            # Mid-attention fillers MUST stay on the dedicated "fil" tag:
            # an alloc on an attention tag would reuse (and reset) a PSUM
            # bank whose ctx/logits accumulation is still in flight.
            qkv_tags = ["fil", "fil", "lgX", "lgY", "ctxX", "ctxY"]
            qkv_bufs = {"fil": 2, "lgX": 1, "lgY": 1, "ctxX": 1, "ctxY": 1}
            qkv_ctr = [0]
            prologue = [True]

            def qkv_tag():
                if not prologue[0]:
                    return "fil"
                t = qkv_tags[qkv_ctr[0] % len(qkv_tags)]
                qkv_ctr[0] += 1
                return t

            def emit_qk_group(p, qc, which):
                _t = qkv_tag()
                ps = pp.tile([128, TQ], F32, tag=_t, bufs=qkv_bufs[_t],
                             name=f"ps{which}_{p}_{qc}")
                w_sb = wq_sb if which == "q" else wk_sb
                for d in range(NDT):
                    nc.tensor.matmul(
                        ps[:],
                        lhsT=w_sb[:, d * 256 + 128 * p:d * 256 + 128 * p + 128],
                        rhs=xt_ap(d, qc * TQ, (qc + 1) * TQ),
                        start=(d == 0), stop=(d == NDT - 1))
                dst = qT_sb if which == "q" else kT_sb
                b_sb = bq_sb if which == "q" else bk_sb
                nc.vector.tensor_scalar_add(
                    dst[:, p * T + qc * TQ:p * T + (qc + 1) * TQ],
                    ps[:], b_sb[:, p:p + 1])

            def emit_v_group(tt):
                _t = qkv_tag()
                psv = pp.tile([128, WV], F32, tag=_t, bufs=qkv_bufs[_t],
                              name=f"psv_{tt}")
                for d in range(NDT):
                    nc.tensor.matmul(
                        psv[:],
                        lhsT=xt_ap(d, tt * 128, (tt + 1) * 128),
                        rhs=wv_sb[:, d * WV:(d + 1) * WV],
                        start=(d == 0), stop=(d == NDT - 1))
                nc.vector.tensor_add(v_sb[:, tt * WV:(tt + 1) * WV],
                                     psv[:], bv_sb[:])

            filler = []

            def pump(n):
                for _ in range(min(n, len(filler))):
                    filler.pop(0)()

            # ---------------- attention: one head pair, one q-chunk -------
            def emit_attn_pair(pair, qc):
                # chain X = head 2*pair (partition rows 0-63 of kT/qT p-tile
                # `pair`), chain Y = head 2*pair+1 (rows 64-127); K=64 logits
                # matmuls occupy disjoint PE row-groups and run concurrently.
                nkt = 4 * qc + 4
                ctxs = {}
                for grp in range(nkt // 2):
                    lgs = {}
                    exs = {}
                    for cn in ("X", "Y"):
                        if grp == 0:
                            ctxs[cn] = pp.tile([VW, TQ], F32, tag=f"ctx{cn}",
                                               bufs=1, name=f"ctx_{pair}{cn}_{qc}")
                        lgs[cn] = pp.tile([128, 2 * TQ], F32, tag=f"lg{cn}",
                                          bufs=1,
                                          name=f"lg_{pair}{cn}_{qc}_{grp}")
                        exs[cn] = sp.tile([128, 2 * TQ], BF16, tag=f"ex{cn}",
                                          bufs=5, name=f"ex_{pair}{cn}_{qc}_{grp}")
                    for j in range(2):
                        kt = 2 * grp + j
                        for half, cn in ((0, "X"), (1, "Y")):
                            r0 = DK * half
                            nc.tensor.matmul(
                                lgs[cn][:, j * TQ:(j + 1) * TQ],
                                lhsT=kT_sb[r0:r0 + DK,
                                           pair * T + kt * 128:
                                           pair * T + (kt + 1) * 128],
                                rhs=qT_sb[r0:r0 + DK,
                                          pair * T + qc * TQ:
                                          pair * T + (qc + 1) * TQ],
                                start=True, stop=True)
                    for cn in ("X", "Y"):
                        nc.scalar.activation(exs[cn][:], lgs[cn][:], AF.Exp,
                                             scale=SCALE)
                    # diagonal band: kts [4qc, 4qc+4) live in grps 2qc, 2qc+1
                    # exactly; apply both kt halves' masks in one op.
                    if grp >= 2 * qc:
                        r = 2 * (grp - 2 * qc)
                        for cn in ("X", "Y"):
                            nc.vector.tensor_mul(
                                exs[cn][:],
                                exs[cn][:],
                                mask_sb[:, r * TQ:(r + 2) * TQ])
                    for half, cn in ((0, "X"), (1, "Y")):
                        h = 2 * pair + half
                        for j in range(2):
                            kt = 2 * grp + j
                            nc.tensor.matmul(
                                ctxs[cn][:],
                                lhsT=v_sb[:, kt * WV + VW * h:
                                          kt * WV + VW * h + VW],
                                rhs=exs[cn][:, j * TQ:(j + 1) * TQ],
                                start=(kt == 0), stop=(kt == nkt - 1))
                    pump(1)
                # normalize: ctx[0:64] * (1/ctx[64]) -> ctxn (bf16, SBUF).
                # gpsimd must stay collective-only (its queue blocks on
                # in-flight RS), so the partition broadcast of the
                # reciprocal runs on the PE as a rank-1 matmul.
                for half, cn in ((0, "X"), (1, "Y")):
                    ctx = ctxs[cn]
                    # custom-DVE ops can't take a shifted partition base:
                    # stage the denominator row at partition 0 first.
                    dn = sp.tile([1, TQ], F32, tag=f"dn{cn}", bufs=2,
                                 name=f"dn_{pair}{cn}_{qc}")
                    nc.vector.tensor_copy(dn[:], ctx[DK:DK + 1, :])
                    ctxu = sp.tile([DK, TQ], F32, tag=f"ctxu{cn}", bufs=2,
                                   name=f"ctxu_{pair}{cn}_{qc}")
                    nc.vector.tensor_copy(ctxu[:], ctx[0:DK, :])
                    rc = sp.tile([1, TQ], F32, tag=f"rc{cn}", bufs=2,
                                 name=f"rc_{pair}{cn}_{qc}")
                    nc.vector.reciprocal_approx_fast(rc[:], dn[:])
                    rcb = pp.tile([DK, TQ], F32, tag="fil", bufs=2,
                                  name=f"rcb_{pair}{cn}_{qc}")
                    nc.tensor.matmul(rcb[:], lhsT=ones1[:], rhs=rc[:],
                                     start=True, stop=True)
                    nc.vector.tensor_mul(
                        ctxn[pair][DK * half:DK * half + DK,
                                   qc * TQ:(qc + 1) * TQ],
                        ctxu[:], rcb[:])

            # ---------------- output projection partial + RS --------------
            def emit_proj(qc):
                # partial[t, o] = sum_c ctxn[c, t] * wo[c, o] over local 256 c
                for tt in range(4):
                    t0 = qc * TQ + tt * 128
                    pa = sp.tile([128, D], BF16, tag="pa", bufs=3,
                                 name=f"pa_{qc}_{tt}")
                    for oc in range(2):
                        po = pp.tile([128, TQ], F32, tag="fil", bufs=2,
                                     name=f"po_{qc}_{tt}_{oc}")
                        for p in range(2):
                            nc.tensor.matmul(
                                po[:],
                                lhsT=ctxn[p][:, t0:t0 + 128],
                                rhs=wo_sb[:, p * D + oc * TQ:
                                          p * D + (oc + 1) * TQ],
                                start=(p == 0), stop=(p == 1))
                        nc.vector.tensor_copy(pa[:, oc * TQ:(oc + 1) * TQ],
                                              po[:])
                    nc.sync.dma_start(partial[t0:t0 + 128, :], pa[:])

            def emit_rs(qc):
                nc.gpsimd.collective_compute(
                    "ReduceScatter",
                    mybir.AluOpType.add,
                    replica_groups=[[0, 1, 2, 3], [4, 5, 6, 7]],
                    ins=[partial[qc * TQ:(qc + 1) * TQ, :].opt()],
                    outs=[rs_out[qc][:, :].opt()],
                )

            def emit_rs_post(qc):
                # only emitted once RS(qc) is long done -> no queue stall.
                rsb = sp.tile([128, D], BF16, tag="rsb", bufs=2,
                              name=f"rsb_{qc}")
                for i in range(2):
                    nc.sync.dma_start(rsb[:, i * TQ:(i + 1) * TQ],
                                      rs_out[qc][:, i * TQ:(i + 1) * TQ])
                osb = sp.tile([128, D], F32, tag="osb", bufs=2,
                              name=f"osb_{qc}")
                nc.vector.tensor_add(osb[:], rsb[:], bo_sb[:])
                for i in range(4):
                    nc.sync.dma_start(
                        out[qc * 128:(qc + 1) * 128, i * 256:(i + 1) * 256],
                        osb[:, i * 256:(i + 1) * 256])

            # ---------------- schedule ----------------
            # prologue: QKV chunks needed by qc=0 attention
            emit_qk_group(0, 0, "q")
            emit_qk_group(0, 0, "k")
            for tt in range(4):
                emit_v_group(tt)
            emit_qk_group(1, 0, "q")
            emit_qk_group(1, 0, "k")
            prologue[0] = False

            for qc in range(NQC):
                # enqueue next qc's QKV as fillers
                if qc + 1 < NQC:
                    nqc = qc + 1
                    filler.append(lambda p=0, q=nqc: emit_qk_group(p, q, "q"))
                    filler.append(lambda p=0, q=nqc: emit_qk_group(p, q, "k"))
                    for tt in range(4 * nqc, 4 * nqc + 4):
                        filler.append(lambda t=tt: emit_v_group(t))
                    filler.append(lambda p=1, q=nqc: emit_qk_group(p, q, "q"))
                    filler.append(lambda p=1, q=nqc: emit_qk_group(p, q, "k"))
                emit_attn_pair(0, qc)
                emit_attn_pair(1, qc)
                pump(2)
                emit_proj(qc)
                emit_rs(qc)
                if qc >= 2:
                    emit_rs_post(qc - 2)
            pump(len(filler))
            emit_rs_post(2)
            emit_rs_post(3)

            if DEBUG:
                nc.sync.dma_start(dbg["dbg_qT"][:], qT_sb[:])
                nc.sync.dma_start(dbg["dbg_kT"][:], kT_sb[:])
                nc.sync.dma_start(dbg["dbg_v"][:], v_sb[:])
                nc.sync.dma_start(dbg["dbg_ctxn0"][:], ctxn[0][:])
                nc.sync.dma_start(dbg["dbg_ctxn1"][:], ctxn[1][:])
                nc.sync.dma_start(dbg["dbg_partial"][:], partial[:])
                for q in range(NQC):
                    nc.sync.dma_start(
                        dbg["dbg_rs"][q * 128:(q + 1) * 128, :],
                        rs_out[q][:])

    nc.compile()
    return nc


def _masks_np():
    jj = np.arange(128)[:, None]
    ii = np.arange(TQ)[None, :]
    m = np.zeros((128, 4 * TQ), np.float32)
    for r in range(4):
        m[:, r * TQ:(r + 1) * TQ] = (jj + 128 * r <= ii)
    return m.astype(ml_dtypes.bfloat16)


def _shard_inputs(x, Wqkv, bqkv, Wo, bo_v):
    bf = ml_dtypes.bfloat16
    masks = _masks_np()
    in_maps = []
    for c in range(NCORES):
        b, g = c // 4, c % 4
        h0 = 4 * g
        q0 = h0 * DK
        # xt: [128, qc*4096 + d*512 + c] = x[b][qc*512+c, 128d+p]
        xb = np.asarray(x[b])  # [T, D]
        xt = np.ascontiguousarray(
            xb.reshape(NQC, TQ, NDT, 128).transpose(3, 0, 2, 1)
            .reshape(128, NQC * NDT * TQ)).astype(bf)
        # wq/wk: [128, d*256 + j] = W[128d+p, q0+j]
        wq = np.ascontiguousarray(
            Wqkv[:, q0:q0 + 256].reshape(NDT, 128, 256).transpose(1, 0, 2)
            .reshape(128, NDT * 256)).astype(bf)
        wk = np.ascontiguousarray(
            Wqkv[:, D + q0:D + q0 + 256].reshape(NDT, 128, 256)
            .transpose(1, 0, 2).reshape(128, NDT * 256)).astype(bf)
        # wv: [128, d*260 + 65j + cc] = Wv[128d+p, (h0+j)*64+cc], ones col 0
        wv_flat = np.zeros((D, WV), np.float32)
        bv = np.zeros((WV,), np.float32)
        for j in range(HPC):
            wv_flat[:, VW * j:VW * j + DK] = Wqkv[:, 2 * D + (h0 + j) * DK:
                                                  2 * D + (h0 + j + 1) * DK]
            bv[VW * j:VW * j + DK] = bqkv[2 * D + (h0 + j) * DK:
                                          2 * D + (h0 + j + 1) * DK]
            bv[VW * j + DK] = 1.0
        wv = np.ascontiguousarray(
            wv_flat.reshape(NDT, 128, WV).transpose(1, 0, 2)
            .reshape(128, NDT * WV)).astype(bf)
        # wo: [128, pair*1024 + o] = Wo[(4g+2*pair)*64 + p, o]
        wo = np.ascontiguousarray(np.concatenate(
            [Wo[(h0 + 2 * p) * DK:(h0 + 2 * p) * DK + 128, :]
             for p in range(2)], axis=1)).astype(bf)
        in_maps.append({
            "xt": xt,
            "wq": wq,
            "wk": wk,
            "wv": wv,
            "wo": wo,
            "bq": np.stack([bqkv[q0:q0 + 128], bqkv[q0 + 128:q0 + 256]],
                           axis=1).astype(np.float32).copy(),
            "bk": np.stack([bqkv[D + q0:D + q0 + 128],
                            bqkv[D + q0 + 128:D + q0 + 256]],
                           axis=1).astype(np.float32).copy(),
            "bv": np.ascontiguousarray(
                np.broadcast_to(bv, (128, WV))).astype(np.float32),
            "bo": np.ascontiguousarray(
                np.broadcast_to(bo_v, (128, D))).astype(np.float32),
            "masks": masks,
        })
    return in_maps


def kernel(**inputs):
    x = np.asarray(inputs["x"], np.float32)
    Wqkv = np.asarray(inputs["W_qkv"], np.float32)
    bqkv = np.asarray(inputs["b_qkv"], np.float32)
    Wo = np.asarray(inputs["W_o"], np.float32)
    bo_v = np.asarray(inputs["b_o"], np.float32)

    if "nc" not in _cache:
        _cache["nc"] = _build()
    nc = _cache["nc"]

    in_maps = _shard_inputs(x, Wqkv, bqkv, Wo, bo_v)
    res = bass_utils.run_bass_kernel_spmd(
        nc, in_maps, core_ids=list(range(NCORES)), trace=TRACE)
    LAST_RESULT["exec_time_ns"] = res.exec_time_ns
    LAST_RESULT["res"] = res

    out = np.empty((2, T, D), np.float32)
    for c in range(NCORES):
        b, g = c // 4, c % 4
        co = res.results[c]["out"]  # [512, 1024]
        for qc in range(NQC):
            out[b, qc * TQ + 128 * g:qc * TQ + 128 * (g + 1), :] = \
                co[qc * 128:(qc + 1) * 128, :]
    return out
